# revision 21
# baseline (speedup 1.0000x reference)
"""Mixtral-style MoE (T=2048, H=2048, I=7168, E=8, top_k=2) on 8 trn2 cores.

Strategy: expert parallelism. Host computes the (tiny) router in float64,
gathers each expert's tokens, and pre-lays-out that expert's weights so that
every device DMA is contiguous-per-partition. Core e computes
    gT = silu(w1[e] @ x_eT) * (w3[e] @ x_eT)        # [I, C] via float32r matmuls
    out_e = (gT.T @ w2[e].T) * route_weight[:,None]  # [C, H] via bf16 matmuls
Host scatters the 8 per-expert outputs back into the full [T, H] output.
"""

import sys

import numpy as np

for _p in ("/opt/trn_rl_repo", "/root/.axon_site/_ro/trn_rl_repo"):
    if _p not in sys.path:
        sys.path.insert(0, _p)

import ml_dtypes  # noqa: E402

P = 128


# ---------------------------------------------------------------- host routing
def _route(hs, gw, top_k):
    """float64 softmax router; returns sel [T,k] int, rw [T,k] float32."""
    logits = hs.astype(np.float64) @ gw.astype(np.float64).T  # [T, E]
    z = logits - logits.max(axis=-1, keepdims=True)
    p = np.exp(z)
    p /= p.sum(axis=-1, keepdims=True)
    # top-k indices (order within top-k irrelevant: weights are renormalized)
    sel = np.argpartition(-p, kth=top_k - 1, axis=-1)[:, :top_k]
    rw = np.take_along_axis(p, sel, axis=-1)
    rw = rw / rw.sum(axis=-1, keepdims=True)
    return sel, rw.astype(np.float32)


# ------------------------------------------------------------- device program
_PROGRAM_CACHE = {}


def _build_program(C, H, I, hbw=256, reps=1):
    """Build the SPMD Bass program for one expert with capacity C tokens.

    reps>1 repeats the whole computation in-NEFF (used only for timing:
    the delta between rep counts isolates pure HW execution time)."""
    key = (C, H, I, hbw, reps)
    if key in _PROGRAM_CACHE:
        return _PROGRAM_CACHE[key]
    from concourse import bacc, tile
    import concourse.mybir as mybir

    f32 = mybir.dt.float32
    f32r = mybir.dt.float32r
    bf16 = mybir.dt.bfloat16

    KH = H // P          # contraction tiles for phase 1
    NM = I // P          # output row-tiles for phase 1 / contraction tiles ph2
    HB = H // hbw        # output col-blocks for phase 2
    # moving-dim chunks: each <=512 (fp32 moving-operand / PSUM-bank limit),
    # as equal as possible (>=256 keeps float32r at 1 cycle/row)
    def _chunks(total, maxw=512):
        nch = -(-total // maxw)
        # chunk starts aligned to 8 elements (32B) for ISA-legal AP offsets
        bounds = [min(((total * i // nch + 7) // 8) * 8, total) for i in range(nch)]
        bounds.append(total)
        return [(bounds[i], bounds[i + 1] - bounds[i]) for i in range(nch)]

    chunks = _chunks(C)
    chunks2 = _chunks(C)

    nc = bacc.Bacc("TRN2", target_bir_lowering=False, debug=False, num_devices=8)

    xt_d = nc.dram_tensor("xt", [P, KH * C], f32r, kind="ExternalInput").ap()
    w1_d = nc.dram_tensor("w1r", [NM, P, KH * P], f32r, kind="ExternalInput").ap()
    w3_d = nc.dram_tensor("w3r", [NM, P, KH * P], f32r, kind="ExternalInput").ap()
    w2_d = nc.dram_tensor("w2r", [HB, P, NM * hbw], bf16, kind="ExternalInput").ap()
    sc_d = nc.dram_tensor("scale", [P, C], f32, kind="ExternalInput").ap()
    out_d = nc.dram_tensor("out", [H, C], f32, kind="ExternalOutput").ap()

    NQ = 4 if NM % 4 == 0 else 1
    QW = NM // NQ

    with tile.TileContext(nc) as tc:
        with (
            tc.tile_pool(name="persist", bufs=1) as persist,
            tc.tile_pool(name="slab0", bufs=1) as slab0p,
        ):
            sc_sb = persist.tile([P, C], f32)
            g_sb = persist.tile([P, NM * C], bf16)

            def one_rep():
                # ------------- phase 1: gT[m*P+p, c] in SBUF (bf16) ---------
                with (
                    tc.tile_pool(name="xtp", bufs=1) as xtp,
                    tc.tile_pool(name="wblk", bufs=4) as wblk,
                    tc.tile_pool(name="ev1", bufs=3) as ev1,
                    tc.tile_pool(name="ps1", bufs=2, space="PSUM") as ps1,
                ):
                    xt_tiles = None
                    for m in range(NM):
                        w1_sb = wblk.tile([P, KH * P], f32r, tag="w")
                        nc.sync.dma_start(w1_sb[:], w1_d[m])
                        w3_sb = wblk.tile([P, KH * P], f32r, tag="w")
                        nc.sync.dma_start(w3_sb[:], w3_d[m])
                        if xt_tiles is None:
                            # per-k xt tiles, emitted after m=0's weights so
                            # the first matmul only waits for w1[0] + xt[0]
                            xt_tiles = []
                            for k in range(KH):
                                xk = xtp.tile([P, C], f32r, tag=f"xt{k}",
                                              name=f"xt{k}")
                                nc.sync.dma_start(
                                    xk[:], xt_d[:, k * C : (k + 1) * C]
                                )
                                xt_tiles.append(xk)
                        if m == NM - 1:
                            # prefetch phase-2 oddments during the phase-1 tail
                            nc.sync.dma_start(sc_sb[:], sc_d[:])
                            if NQ == 4:
                                s0 = slab0p.tile([P, QW * hbw], bf16, name="s0")
                                nc.sync.dma_start(s0[:], w2_d[0][:, : QW * hbw])
                        for c0, cw in chunks:
                            y1 = ps1.tile([P, cw], f32, tag="y1")
                            y3 = ps1.tile([P, cw], f32, tag="y3")
                            for k in range(KH):
                                lhs1 = w1_sb[:, k * P : (k + 1) * P]
                                lhs3 = w3_sb[:, k * P : (k + 1) * P]
                                rhs = xt_tiles[k][:, c0 : c0 + cw]
                                nc.tensor.matmul(
                                    y1[:], lhs1, rhs, start=(k == 0), stop=(k == KH - 1)
                                )
                                nc.tensor.matmul(
                                    y3[:], lhs3, rhs, start=(k == 0), stop=(k == KH - 1)
                                )
                            gt = ev1.tile([P, cw], f32, tag="gt")
                            nc.scalar.activation(
                                gt[:], y1[:], mybir.ActivationFunctionType.Sigmoid
                            )
                            gt2 = ev1.tile([P, cw], f32, tag="gt2")
                            nc.vector.tensor_mul(gt2[:], gt[:], y1[:])
                            gout = g_sb[:, m * C + c0 : m * C + c0 + cw]
                            nc.vector.tensor_mul(gout, gt2[:], y3[:])

                # ---- phase 2: outT[h, t] = w2T.T @ gT, scaled by token ----
                with (
                    tc.tile_pool(name="slab", bufs=2) as slab_pool,
                    tc.tile_pool(name="ev2", bufs=3) as ev2,
                    tc.tile_pool(name="ps2", bufs=3, space="PSUM") as ps2,
                ):
                    for hb in range(HB):
                        # quarter-split the slab DMA so phase-2 matmuls can
                        # start before the whole h-block's weights land
                        slabs = []
                        for q in range(NQ):
                            if hb == 0 and q == 0 and NQ == 4:
                                slabs.append(s0)
                                continue
                            sq = slab_pool.tile([P, QW * hbw], bf16, tag=f"w2q{q}",
                                                name=f"w2q{q}_{hb}")
                            nc.sync.dma_start(
                                sq[:], w2_d[hb][:, q * QW * hbw : (q + 1) * QW * hbw]
                            )
                            slabs.append(sq)
                        for hl in range(hbw // P):
                            pos = []
                            for j, (c0, cw) in enumerate(chunks2):
                                po_t = ps2.tile([P, cw], f32, tag=f"po{j}",
                                                name=f"po{j}_{hb}_{hl}")
                                pos.append(po_t)
                            for km in range(NM):
                                lhs = slabs[km // QW][
                                    :,
                                    (km % QW) * hbw + hl * P :
                                    (km % QW) * hbw + (hl + 1) * P,
                                ]
                                for j, (c0, cw) in enumerate(chunks2):
                                    rhs = g_sb[:, km * C + c0 : km * C + c0 + cw]
                                    nc.tensor.matmul(
                                        pos[j][:], lhs, rhs,
                                        start=(km == 0), stop=(km == NM - 1),
                                    )
                            for j, (c0, cw) in enumerate(chunks2):
                                osb = ev2.tile([P, cw], f32, tag=f"osb{j}")
                                nc.vector.tensor_mul(
                                    osb[:], pos[j][:], sc_sb[:, c0 : c0 + cw]
                                )
                                nc.sync.dma_start(
                                    out_d[
                                        hb * hbw + hl * P : hb * hbw + (hl + 1) * P,
                                        c0 : c0 + cw,
                                    ],
                                    osb[:],
                                )

            for _rep in range(reps):
                one_rep()

    nc.compile()
    _PROGRAM_CACHE[key] = nc
    return nc


# ------------------------------------------------------------------ host prep
def _prep_core_inputs(hs, w1_e, w3_e, w2_e, idx, wts, C, H, I, hbw=256):
    KH = H // P
    NM = I // P
    HB = H // hbw
    n = len(idx)

    xg = np.zeros((C, H), dtype=np.float32)
    xg[:n] = hs[idx]
    xt = np.ascontiguousarray(xg.T).reshape(KH, P, C).transpose(1, 0, 2)
    xt = np.ascontiguousarray(xt).reshape(P, KH * C)

    w1r = np.ascontiguousarray(
        w1_e.reshape(NM, P, KH, P).transpose(0, 3, 2, 1)
    ).reshape(NM, P, KH * P)
    w3r = np.ascontiguousarray(
        w3_e.reshape(NM, P, KH, P).transpose(0, 3, 2, 1)
    ).reshape(NM, P, KH * P)
    w2r = np.ascontiguousarray(
        w2_e.astype(ml_dtypes.bfloat16).reshape(HB, hbw, NM, P).transpose(0, 3, 2, 1)
    ).reshape(HB, P, NM * hbw)

    sc1 = np.zeros(C, dtype=np.float32)
    sc1[:n] = wts
    sc = np.ascontiguousarray(np.broadcast_to(sc1[None, :], (P, C)))

    return {"xt": xt, "w1r": w1r, "w3r": w3r, "w2r": w2r, "scale": sc}


# ---------------------------------------------------------------------- entry
def _run(inputs, trace=False, trace_cores=None):
    from concourse.bass_utils import run_bass_kernel_spmd

    hs = np.asarray(inputs["hidden_states"], dtype=np.float32)
    gw = np.asarray(inputs["gate_w"], dtype=np.float32)
    w1 = np.asarray(inputs["w1"], dtype=np.float32)
    w3 = np.asarray(inputs["w3"], dtype=np.float32)
    w2 = np.asarray(inputs["w2"], dtype=np.float32)
    top_k = int(np.asarray(inputs["top_k"]))

    T, H = hs.shape
    E, I, _ = w1.shape
    n_cores = E  # one expert per core

    sel, rw = _route(hs, gw, top_k)

    idxs, wtss = [], []
    for e in range(E):
        mask = sel == e  # [T, k]
        tok = np.nonzero(mask.any(axis=-1))[0]
        wt = rw[mask]  # in token order since mask rows have <=1 True
        idxs.append(tok)
        wtss.append(wt)

    cmax = max(len(i) for i in idxs)
    C = max(((cmax + 1) // 2) * 2, P)  # even, no further padding needed
    hbw = 256

    nc = _build_program(C, H, I, hbw=hbw)

    in_maps = [
        _prep_core_inputs(hs, w1[e], w3[e], w2[e], idxs[e], wtss[e], C, H, I, hbw=hbw)
        for e in range(E)
    ]

    res = run_bass_kernel_spmd(
        nc,
        in_maps,
        list(range(n_cores)),
        trace=trace,
        **({"trace_cores": trace_cores} if trace_cores is not None else {}),
    )

    out = np.zeros((T, H), dtype=np.float32)
    for e in range(E):
        n = len(idxs[e])
        out[idxs[e]] += res.results[e]["out"].T[:n]
    return out, res


def kernel(**inputs):
    return _run(inputs, trace=False)[0]
